# revision 18
# baseline (speedup 1.0000x reference)
"""Single-head attention on 8 Trainium2 NeuronCores (v3).

Problem: x[8, 2048, 768], Wq/Wk/Wv[768, 64]+biases, mask[2048, 2048] int32
Output:  softmax(mask(Q K^T / 8)) V   -> [8, 2048, 64] f32

Sharding: data-parallel over batch - core b computes batch element b.

Per-core dataflow (all matmuls bf16 in / fp32 psum accumulate):
  proj:  QK[128, n] = Wqk.T @ xT per 512-col group; bias added as a k=1
         matmul (lhsT=bias row, rhs=ones row). Q copied to both partition
         halves of qq; K scattered into kk by chunk parity (even chunks at
         partitions 0:64, odd at 64:128) so K needs no duplication for the
         row-tiled score matmuls. V as [keys,64] + ones column -> vp, so the
         PV matmul also yields the softmax denominator for free.
  loop:  per (sweep s of 1024 q-cols, key chunk k): one row-tiled score
         matmul pair -> st psum, one [128,1024] exp on ACT (the critical
         engine: ~33us total at 1 col/cycle), mask multiply on DVE (bf16 2x
         mode), two PV matmuls accumulating OT[65, 1024]; PV for chunk k-1
         is emitted alongside scores for chunk k so the PE stream never
         stalls on the exp/mult latency. Remaining projections are wedged
         into sweep 0's early chunks.
  out:   OT psum -> sbuf -> DMA; host does out[q,h] = OT[h,q]/OT[64,q].

DMAs all ride the SP hardware-DGE queue in consumption order; the mask is
moved in 8 big strided DMAs (4 chunks x 1024 cols each) to keep the issue
cost low (~565ns per DMA on the SP sequencer).
"""

import os

import numpy as np
import ml_dtypes

import bass_rust
import concourse.bass as bass
import concourse.mybir as mybir
import concourse.tile as tile
from concourse.bass_utils import run_bass_kernel_spmd

BF16 = ml_dtypes.bfloat16
F32 = mybir.dt.float32
BF = mybir.dt.bfloat16

N_CORES = 8
SEQ = 2048
WIDTH = 768
HEAD = 64
NCH = WIDTH // 128      # 6 contraction chunks for the projections
NKC = SEQ // 128        # 16 key chunks
QT = 1024               # q tile (columns processed per main-loop sweep)
NQT = SEQ // QT


def _split_excess_waits(nc, max_waits=1):
    """walrus in this container rejects >1 sync wait per instruction; hoist
    extras onto preceding same-engine NoOps (same semantics: the engine
    executes its stream in order, so waiting earlier is equivalent)."""
    n = 0
    for bb in nc.main_func.blocks:
        new_list = []
        for ins in bb.instructions:
            si = ins.sync_info
            if si is not None and len(si.on_wait) > max_waits:
                waits = list(si.on_wait)
                extra, keep = waits[:-max_waits], waits[-max_waits:]
                for j, w in enumerate(extra):
                    nop = bass_rust.InstNoOp(
                        name=f"{ins.name}-ws{j}", engine=ins.engine, ins=[], outs=[]
                    )
                    nop.sync_info = mybir.SyncInfo(on_wait=[w], on_update=[])
                    new_list.append(nop)
                    n += 1
                ins.sync_info = mybir.SyncInfo(
                    on_wait=keep, on_update=list(si.on_update)
                )
            new_list.append(ins)
        bb.instructions = new_list
    return n


def _strip_tail(nc):
    """Drop the NRT pseudo-sync ISA op and the second all-engine barrier that
    TileContext emits after the semaphore reset - ~4-5us of fixed tail. The
    final DMA-drain + first barrier + sem reset are kept, so re-execution of
    the NEFF still starts from clean semaphores."""
    for bb in nc.main_func.blocks:
        ins_list = list(bb.instructions)
        idx = None
        for i, ins in enumerate(ins_list):
            if getattr(ins, "is_reset_sema", False):
                idx = i
        if idx is not None and idx > len(ins_list) - 20:
            bb.instructions = ins_list[:idx + 1]
    return nc




def _hoist_dmas(nc):
    """Move wait-free SP-queue input-DMA issues to the front of the program.
    The TileContext/Bass entry sequence (two all-engine barriers, register
    restores) costs ~7.5us before the first DMA would otherwise issue; the
    input DMAs have no dependencies, and per-engine instruction order is all
    that codegen cares about, so issuing them first lets the transfers run
    during the preamble. DMA-completion semaphores only ever increase and
    consumers wait on >= thresholds, so early completion is harmless."""
    n = 0
    for bb in nc.main_func.blocks:
        front, rest = [], []
        stopped = False
        for ins in bb.instructions:
            si = ins.sync_info
            is_sp_dma = (type(ins).__name__ == "InstDMACopy"
                         and ins.engine == mybir.EngineType.SP)
            if is_sp_dma and stopped is False:
                if si is None or len(si.on_wait) == 0:
                    # Only a leading prefix of wait-free DMAs may move: DMAs
                    # sharing a ring semaphore must complete in queue order,
                    # so nothing may be hoisted past a waiting DMA.
                    front.append(ins)
                    n += 1
                    continue
                stopped = True
            rest.append(ins)
        bb.instructions = front + rest
    return n


def _build():
    nc = bass.Bass("TRN2", target_bir_lowering=False, debug=False,
                   num_devices=N_CORES)

    # partition-major host layouts: row p holds everything partition p needs,
    # so each DMA is 128 large contiguous descriptors.
    xT_d = nc.declare_dram_parameter("xT", [128, 4 * NCH * 512], BF, False).ap()
    wqk_d = nc.declare_dram_parameter("Wqk", [128, NCH * 128], BF, False).ap()
    wv_d = nc.declare_dram_parameter("Wv", [128, NCH * HEAD], BF, False).ap()
    bqk_d = nc.declare_dram_parameter("bqk", [1, 128], BF, False).ap()
    # mT layout: [p][(s*NKC + c)*1024 + j] = mask[s*1024+j, c*128+p]
    mT_d = nc.declare_dram_parameter("mT", [128, NKC * SEQ], BF, False).ap()
    ot_d = nc.declare_dram_parameter("ot", [HEAD + 1, SEQ], F32, True).ap()

    EXP = mybir.ActivationFunctionType.Exp
    COPY = mybir.ActivationFunctionType.Copy
    ESCALE = 0.125   # 1/sqrt(HEAD)

    with tile.TileContext(nc) as tc:
        with (
            tc.tile_pool(name="const", bufs=1) as const,
            tc.tile_pool(name="pp", bufs=4) as ppool,
            tc.tile_pool(name="ep", bufs=2) as epool,
            tc.tile_pool(name="stp", bufs=3, space="PSUM") as stp,
            tc.tile_pool(name="otp", bufs=1, space="PSUM") as otp,
        ):
            # ---- input DMAs, all SP queue, in consumption order ----
            wqk = const.tile([128, NCH, 128], BF)
            bqk = const.tile([1, 128], BF)
            wv = const.tile([128, NCH, HEAD], BF)
            xt = const.tile([128, 4, NCH, 512], BF)
            mt = const.tile([128, NKC, SEQ], BF)

            def xt_dma(t):
                nc.sync.dma_start(
                    out=xt[:, t, :, :],
                    in_=xT_d[:, t * NCH * 512:(t + 1) * NCH * 512],
                )

            def mask_dma(s, c0, nck):
                # chunks c0..c0+nck of sweep s: contiguous in DRAM, strided
                # into mt (2KB per descriptor row).
                src0 = (s * NKC + c0) * QT
                nc.sync.dma_start(
                    out=mt[:, c0:c0 + nck, s * QT:(s + 1) * QT],
                    in_=mT_d[:, src0:src0 + nck * QT],
                )

            nc.sync.dma_start(out=wqk, in_=wqk_d)
            nc.sync.dma_start(out=bqk, in_=bqk_d)
            xt_dma(0)
            xt_dma(1)
            nc.sync.dma_start(out=wv, in_=wv_d)
            mask_dma(0, 0, 4)
            xt_dma(2)
            xt_dma(3)
            mask_dma(0, 4, 4)
            mask_dma(0, 8, 4)
            mask_dma(0, 12, 4)
            for c0 in range(0, NKC, 4):
                mask_dma(1, c0, 4)

            ones = const.tile([1, 512], BF)
            nc.vector.memset(ones, 1.0)

            # ---- projection targets ----
            qq = const.tile([128, SEQ], BF)             # Q on both halves
            kk = const.tile([128, NKC // 2, 128], BF)   # K by chunk parity
            vp = const.tile([128, NKC, HEAD + 1], BF)   # V | ones column
            qktmp = const.tile([128, SEQ], BF)

            def proj_qk(t):
                cols = slice(t * 512, (t + 1) * 512)
                qk_ps = stp.tile([128, 512], F32, tag="st", name=f"qk_ps{t}")
                for c in range(NCH):
                    nc.tensor.matmul(qk_ps, wqk[:, c, :], xt[:, t, c, :],
                                     start=(c == 0), stop=False)
                nc.tensor.matmul(   # +bias: bqk[m] * ones[n]
                    qk_ps, bqk[0:1, :], ones[0:1, 0:512],
                    start=False, stop=True,
                )
                nc.vector.tensor_copy(out=qktmp[:, cols], in_=qk_ps)
                nc.vector.tensor_copy(out=qq[0:64, cols], in_=qktmp[0:64, cols])
                nc.vector.tensor_copy(out=qq[64:128, cols], in_=qktmp[0:64, cols])
                for j in range(4):
                    kc = 4 * t + j          # absolute key chunk
                    half = slice(0, 64) if kc % 2 == 0 else slice(64, 128)
                    nc.vector.tensor_copy(
                        out=kk[half, kc // 2, :],
                        in_=qktmp[64:128, t * 512 + j * 128:t * 512 + (j + 1) * 128],
                    )

            def proj_v(t, jlo, jhi):
                v_ps = stp.tile([128, jhi - jlo, HEAD], F32, tag="st",
                                name=f"v_ps{t}_{jlo}")
                for j in range(jlo, jhi):
                    for c in range(NCH):
                        nc.tensor.matmul(
                            v_ps[:, j - jlo, :],
                            xt[:, t, c, j * 128:(j + 1) * 128],
                            wv[:, c, :], start=(c == 0), stop=(c == NCH - 1),
                        )
                lo, hi = 4 * t + jlo, 4 * t + jhi
                nc.vector.tensor_copy(out=vp[:, lo:hi, 0:HEAD], in_=v_ps)
                nc.vector.memset(vp[:, lo:hi, HEAD:HEAD + 1], 1.0)

            def kk_ap(k):
                half = slice(0, 64) if k % 2 == 0 else slice(64, 128)
                return kk[half, k // 2, :]

            def qq_half(k, gq):
                half = slice(0, 64) if k % 2 == 0 else slice(64, 128)
                return qq[half, gq]

            proj_qk(0)
            proj_v(0, 0, 4)

            # ---- main loop ----
            for s in range(NQT):
                ot_ps = otp.tile([HEAD + 1, QT], F32, tag="ot", name=f"ot_ps{s}")
                prev = []
                for k in range(NKC + 1):
                    cur = []
                    if k < NKC:
                        # chunks 0-3 of sweep 0 run in 512-wide halves: the
                        # h0 half depends only on proj group t0, so exp
                        # starts while xT group t1 is still arriving.
                        split = (s == 0 and k < 4)
                        st = stp.tile([128, QT], F32, tag="st", name=f"st{s}_{k}")
                        p = ppool.tile([128, QT], BF, tag="p", name=f"p{s}_{k}")
                        halves = ((0, 512), (512, 1024)) if split else ((0, QT),)
                        for lo, hi in halves:
                            if lo == 512 and s == 0 and k == 0:
                                # the h1 columns need proj group t1: the qk(1)
                                # wedge must be emitted BEFORE any h1 reader,
                                # or the scores read stale qq on a cold run
                                proj_qk(1)
                            for h in range(lo // 512, hi // 512):
                                gq = slice(s * QT + h * 512, s * QT + (h + 1) * 512)
                                nc.tensor.matmul(
                                    st[:, h * 512:(h + 1) * 512],
                                    kk_ap(k), qq_half(k, gq),
                                    start=True, stop=True,
                                )
                            nc.scalar.activation(
                                p[:, lo:hi], st[:, lo:hi], EXP, scale=ESCALE)
                            gqs = slice(s * QT + lo, s * QT + hi)
                            nc.vector.tensor_mul(
                                p[:, lo:hi], p[:, lo:hi], mt[:, k, gqs])
                            for h in range(lo // 512, hi // 512):
                                cur.append((k, p, h))
                        if s == 0:
                            # remaining projection work wedged into the PE
                            # stream while ACT digests the early chunks
                            if k == 0:
                                proj_v(1, 0, 2)
                            elif k == 1:
                                proj_v(1, 2, 4)
                            elif k == 2:
                                proj_qk(2)
                            elif k == 3:
                                proj_v(2, 0, 2)
                            elif k == 4:
                                proj_v(2, 2, 4)
                            elif k == 5:
                                proj_qk(3)
                            elif k == 6:
                                proj_v(3, 0, 2)
                            elif k == 7:
                                proj_v(3, 2, 4)
                    for (pk, p, h) in prev:
                        nc.tensor.matmul(
                            ot_ps[:, h * 512:(h + 1) * 512],
                            vp[:, pk, :], p[:, h * 512:(h + 1) * 512],
                            start=(pk == 0), stop=(pk == NKC - 1),
                        )
                    prev = cur

                # psum -> sbuf, then DMA out. Sweep 0's copy runs mid-loop
                # where ACT is saturated, so it goes on DVE; sweep 1's copy
                # is split ACT/DVE so both halves finish sooner.
                ot_sb = epool.tile([HEAD + 1, QT], F32, tag="osb", name=f"ot_sb{s}")
                if s == 0:
                    nc.vector.tensor_copy(out=ot_sb, in_=ot_ps)
                    nc.sync.dma_start(out=ot_d[:, 0:QT], in_=ot_sb)
                else:
                    nc.scalar.activation(ot_sb[:, 0:512], ot_ps[:, 0:512], COPY)
                    nc.vector.tensor_copy(
                        out=ot_sb[:, 512:1024], in_=ot_ps[:, 512:1024])
                    nc.sync.dma_start(
                        out=ot_d[:, QT:QT + 512], in_=ot_sb[:, 0:512])
                    nc.sync.dma_start(
                        out=ot_d[:, QT + 512:SEQ], in_=ot_sb[:, 512:1024])

    if os.environ.get('ATTN_HOIST', '1') == '1':
        _hoist_dmas(nc)
    _split_excess_waits(nc)
    _strip_tail(nc)
    return nc


_CACHE = {}


def _get_nc():
    if "nc" not in _CACHE:
        _CACHE["nc"] = _build()
    return _CACHE["nc"]


def _prep_in_maps(x, Wq, bq, Wk, bk, Wv, bv, mask):
    x = np.asarray(x, dtype=np.float32)
    Wqk = np.concatenate(
        [np.asarray(Wq, np.float32), np.asarray(Wk, np.float32)], axis=1)
    # partition-major: row p holds [c0 cols | c1 cols | ...] for w = c*128+p
    Wqkh = np.ascontiguousarray(
        Wqk.reshape(NCH, 128, 128).transpose(1, 0, 2).reshape(128, NCH * 128)
    ).astype(BF16)
    Wvh = np.ascontiguousarray(
        np.asarray(Wv, np.float32).reshape(NCH, 128, HEAD)
        .transpose(1, 0, 2).reshape(128, NCH * HEAD)
    ).astype(BF16)
    bqk = np.concatenate(
        [np.asarray(bq, np.float32), np.asarray(bk, np.float32)]
    ).astype(BF16).reshape(1, 128)

    # mTh[p, (s*NKC+c)*1024 + j] = mask[s*1024+j, c*128+p]
    mTh = np.ascontiguousarray(
        np.asarray(mask, np.float32).T.reshape(NKC, 128, NQT, QT)
        .transpose(1, 2, 0, 3).reshape(128, NKC * SEQ)
    ).astype(BF16)
    in_maps = []
    for b in range(N_CORES):
        # xth[p, t, c, j] = x[b][t*512+j, c*128+p]
        xth = np.ascontiguousarray(
            x[b].reshape(4, 512, NCH, 128).transpose(3, 0, 2, 1)
            .reshape(128, 4 * NCH * 512)
        ).astype(BF16)
        in_maps.append({
            "xT": xth, "Wqk": Wqkh, "Wv": Wvh, "bqk": bqk, "mT": mTh,
        })
    return in_maps


def _run(in_maps, trace=False, **kw):
    nc = _get_nc()
    return run_bass_kernel_spmd(nc, in_maps, list(range(N_CORES)), trace=trace, **kw)


def kernel(x, Wq, bq, Wk, bk, Wv, bv, mask):
    in_maps = _prep_in_maps(x, Wq, bq, Wk, bk, Wv, bv, mask)
    res = _run(in_maps)
    out = np.empty((N_CORES, SEQ, HEAD), np.float32)
    bvf = np.asarray(bv, np.float32)   # softmax weights sum to 1, so the V
    for b in range(N_CORES):           # bias is just an additive constant
        ot = np.asarray(res.results[b]["ot"])          # [65, 2048] f32
        out[b] = (ot[:HEAD] / ot[HEAD:HEAD + 1]).T + bvf
    return out


# revision 21
# speedup vs baseline: 1.0189x; 1.0189x over previous
"""Single-head attention on 8 Trainium2 NeuronCores (v3).

Problem: x[8, 2048, 768], Wq/Wk/Wv[768, 64]+biases, mask[2048, 2048] int32
Output:  softmax(mask(Q K^T / 8)) V   -> [8, 2048, 64] f32

Sharding: data-parallel over batch - core b computes batch element b.

Per-core dataflow (all matmuls bf16 in / fp32 psum accumulate):
  proj:  QK[128, n] = Wqk.T @ xT per 512-col group; bias added as a k=1
         matmul (lhsT=bias row, rhs=ones row). Q copied to both partition
         halves of qq; K scattered into kk by chunk parity (even chunks at
         partitions 0:64, odd at 64:128) so K needs no duplication for the
         row-tiled score matmuls. V as [keys,64] + ones column -> vp, so the
         PV matmul also yields the softmax denominator for free.
  loop:  per (sweep s of 1024 q-cols, key chunk k): one row-tiled score
         matmul pair -> st psum, one [128,1024] exp on ACT (the critical
         engine: ~33us total at 1 col/cycle), mask multiply on DVE (bf16 2x
         mode), two PV matmuls accumulating OT[65, 1024]; PV for chunk k-1
         is emitted alongside scores for chunk k so the PE stream never
         stalls on the exp/mult latency. Remaining projections are wedged
         into sweep 0's early chunks.
  out:   OT psum -> sbuf -> DMA; host does out[q,h] = OT[h,q]/OT[64,q].

DMAs all ride the SP hardware-DGE queue in consumption order; the mask is
moved in 8 big strided DMAs (4 chunks x 1024 cols each) to keep the issue
cost low (~565ns per DMA on the SP sequencer).
"""

import os

import numpy as np
import ml_dtypes

import bass_rust
import concourse.bass as bass
import concourse.mybir as mybir
import concourse.tile as tile
from concourse.bass_utils import run_bass_kernel_spmd

BF16 = ml_dtypes.bfloat16
F32 = mybir.dt.float32
BF = mybir.dt.bfloat16

N_CORES = 8
SEQ = 2048
WIDTH = 768
HEAD = 64
NCH = WIDTH // 128      # 6 contraction chunks for the projections
NKC = SEQ // 128        # 16 key chunks
QT = 1024               # q tile (columns processed per main-loop sweep)
NQT = SEQ // QT


def _split_excess_waits(nc, max_waits=1):
    """walrus in this container rejects >1 sync wait per instruction; hoist
    extras onto preceding same-engine NoOps (same semantics: the engine
    executes its stream in order, so waiting earlier is equivalent)."""
    n = 0
    for bb in nc.main_func.blocks:
        new_list = []
        for ins in bb.instructions:
            si = ins.sync_info
            if si is not None and len(si.on_wait) > max_waits:
                waits = list(si.on_wait)
                extra, keep = waits[:-max_waits], waits[-max_waits:]
                for j, w in enumerate(extra):
                    nop = bass_rust.InstNoOp(
                        name=f"{ins.name}-ws{j}", engine=ins.engine, ins=[], outs=[]
                    )
                    nop.sync_info = mybir.SyncInfo(on_wait=[w], on_update=[])
                    new_list.append(nop)
                    n += 1
                ins.sync_info = mybir.SyncInfo(
                    on_wait=keep, on_update=list(si.on_update)
                )
            new_list.append(ins)
        bb.instructions = new_list
    return n


def _strip_tail(nc):
    """Drop the NRT pseudo-sync ISA op and the second all-engine barrier that
    TileContext emits after the semaphore reset - ~4-5us of fixed tail. The
    final DMA-drain + first barrier + sem reset are kept, so re-execution of
    the NEFF still starts from clean semaphores."""
    for bb in nc.main_func.blocks:
        ins_list = list(bb.instructions)
        idx = None
        for i, ins in enumerate(ins_list):
            if getattr(ins, "is_reset_sema", False):
                idx = i
        if idx is not None and idx > len(ins_list) - 20:
            bb.instructions = ins_list[:idx + 1]
    return nc




def _hoist_dmas(nc, max_hoist=5):
    """Move wait-free SP-queue input-DMA issues to the front of the program.
    The TileContext/Bass entry sequence (two all-engine barriers, register
    restores) costs ~7.5us before the first DMA would otherwise issue; the
    input DMAs have no dependencies, and per-engine instruction order is all
    that codegen cares about, so issuing them first lets the transfers run
    during the preamble. DMA-completion semaphores only ever increase and
    consumers wait on >= thresholds, so early completion is harmless."""
    blocks = nc.main_func.blocks
    body = None
    for bb in blocks:
        if any(type(ins).__name__ == "InstDMACopy" for ins in bb.instructions):
            body = bb
            break
    if body is None:
        return 0
    # leading prefix of wait-free SP DMAs from the body block. Only a prefix
    # may move: DMAs sharing a ring semaphore must complete in queue order,
    # so nothing may be hoisted past a waiting DMA.
    front, rest = [], []
    stopped = False
    for ins in body.instructions:
        si = ins.sync_info
        if (not stopped and type(ins).__name__ == "InstDMACopy"
                and ins.engine == mybir.EngineType.SP
                and len(front) < max_hoist):
            if si is None or len(si.on_wait) == 0:
                front.append(ins)
                continue
            stopped = True
        rest.append(ins)
    if not front:
        return 0
    body.instructions = rest
    # insert into the ENTRY block before the first SP instruction (the
    # drain/barrier sequence), so the transfers run during the preamble.
    entry = blocks[0]
    pos = 0
    for i, ins in enumerate(entry.instructions):
        if getattr(ins, "engine", None) == mybir.EngineType.SP:
            pos = i
            break
    entry.instructions = (
        entry.instructions[:pos] + front + entry.instructions[pos:]
    )
    return len(front)


def _build():
    nc = bass.Bass("TRN2", target_bir_lowering=False, debug=False,
                   num_devices=N_CORES)

    # partition-major host layouts: row p holds everything partition p needs,
    # so each DMA is 128 large contiguous descriptors.
    xT_d = nc.declare_dram_parameter("xT", [128, 4 * NCH * 512], BF, False).ap()
    wqk_d = nc.declare_dram_parameter("Wqk", [128, NCH * 128], BF, False).ap()
    wv_d = nc.declare_dram_parameter("Wv", [128, NCH * HEAD], BF, False).ap()
    bqk_d = nc.declare_dram_parameter("bqk", [1, 128], BF, False).ap()
    id_d = nc.declare_dram_parameter("ident", [64, 64], BF, False).ap()
    # mT layout: [p][(s*NKC + c)*1024 + j] = mask[s*1024+j, c*128+p]
    mT_d = nc.declare_dram_parameter("mT", [128, NKC * SEQ], BF, False).ap()
    ot_d = nc.declare_dram_parameter("ot", [HEAD + 1, SEQ], F32, True).ap()

    EXP = mybir.ActivationFunctionType.Exp
    COPY = mybir.ActivationFunctionType.Copy
    ESCALE = 0.125   # 1/sqrt(HEAD)

    with tile.TileContext(nc) as tc:
        with (
            tc.tile_pool(name="const", bufs=1) as const,
            tc.tile_pool(name="pp", bufs=4) as ppool,
            tc.tile_pool(name="ep", bufs=2) as epool,
            tc.tile_pool(name="stp", bufs=3, space="PSUM") as stp,
            tc.tile_pool(name="otp", bufs=1, space="PSUM") as otp,
        ):
            # ---- input DMAs, all SP queue, in consumption order ----
            wqk = const.tile([128, NCH, 128], BF)
            bqk = const.tile([1, 128], BF)
            wv = const.tile([128, NCH, HEAD], BF)
            xt = const.tile([128, 4, NCH, 512], BF)
            mt = const.tile([128, NKC, SEQ], BF)

            def xt_dma(t):
                nc.sync.dma_start(
                    out=xt[:, t, :, :],
                    in_=xT_d[:, t * NCH * 512:(t + 1) * NCH * 512],
                )

            def mask_dma(s, c0, nck):
                # chunks c0..c0+nck of sweep s: contiguous in DRAM, strided
                # into mt (2KB per descriptor row).
                src0 = (s * NKC + c0) * QT
                nc.sync.dma_start(
                    out=mt[:, c0:c0 + nck, s * QT:(s + 1) * QT],
                    in_=mT_d[:, src0:src0 + nck * QT],
                )

            ident = const.tile([64, 64], BF)
            nc.sync.dma_start(out=wqk, in_=wqk_d)
            nc.sync.dma_start(out=bqk, in_=bqk_d)
            nc.sync.dma_start(out=ident, in_=id_d)
            xt_dma(0)
            xt_dma(1)
            nc.sync.dma_start(out=wv, in_=wv_d)
            mask_dma(0, 0, 4)
            xt_dma(2)
            xt_dma(3)
            mask_dma(0, 4, 4)
            mask_dma(0, 8, 4)
            mask_dma(0, 12, 4)
            for c0 in range(0, NKC, 4):
                mask_dma(1, c0, 4)

            ones = const.tile([1, 512], BF)
            nc.vector.memset(ones, 1.0)

            # ---- projection targets ----
            qq = const.tile([128, SEQ], BF)             # Q on both halves
            kk = const.tile([128, NKC // 2, 128], BF)   # K by chunk parity
            vp = const.tile([128, NKC, HEAD + 1], BF)   # V | ones column
            qktmp = const.tile([128, SEQ], BF)

            def proj_qk(t):
                cols = slice(t * 512, (t + 1) * 512)
                qk_ps = stp.tile([128, 512], F32, tag="st", name=f"qk_ps{t}")
                for c in range(NCH):
                    nc.tensor.matmul(qk_ps, wqk[:, c, :], xt[:, t, c, :],
                                     start=(c == 0), stop=False)
                nc.tensor.matmul(   # +bias: bqk[m] * ones[n]
                    qk_ps, bqk[0:1, :], ones[0:1, 0:512],
                    start=False, stop=True,
                )
                nc.vector.tensor_copy(out=qktmp[:, cols], in_=qk_ps)
                nc.vector.tensor_copy(out=qq[0:64, cols], in_=qktmp[0:64, cols])
                nc.vector.tensor_copy(out=qq[64:128, cols], in_=qktmp[0:64, cols])
                for j in range(4):
                    kc = 4 * t + j          # absolute key chunk
                    half = slice(0, 64) if kc % 2 == 0 else slice(64, 128)
                    nc.vector.tensor_copy(
                        out=kk[half, kc // 2, :],
                        in_=qktmp[64:128, t * 512 + j * 128:t * 512 + (j + 1) * 128],
                    )

            vtmp = const.tile([64, SEQ], BF)    # V^T staging

            def proj_vt(t):
                # V^T[h, keys] for key group t: one [64,512] psum, 6 matmuls
                cols = slice(t * 512, (t + 1) * 512)
                vt_ps = stp.tile([64, 512], F32, tag="st", name=f"vt_ps{t}")
                for c in range(NCH):
                    nc.tensor.matmul(vt_ps, wv[:, c, :], xt[:, t, c, :],
                                     start=(c == 0), stop=(c == NCH - 1))
                nc.vector.tensor_copy(out=vtmp[:, cols], in_=vt_ps)

            def proj_vtr(t):
                # transpose V^T -> V[keys, h] chunks on the PE (bf16 psum)
                tp_ps = stp.tile([128, 4, HEAD], BF, tag="st", name=f"tp_ps{t}")
                for j in range(4):
                    nc.tensor.transpose(
                        tp_ps[:, j, :],
                        vtmp[:, t * 512 + j * 128:t * 512 + (j + 1) * 128],
                        ident)
                lo = 4 * t
                nc.vector.tensor_copy(out=vp[:, lo:lo + 4, 0:HEAD], in_=tp_ps)

            def kk_ap(k):
                half = slice(0, 64) if k % 2 == 0 else slice(64, 128)
                return kk[half, k // 2, :]

            def qq_half(k, gq):
                half = slice(0, 64) if k % 2 == 0 else slice(64, 128)
                return qq[half, gq]

            nc.vector.memset(vp[:, :, HEAD:HEAD + 1], 1.0)
            proj_qk(0)

            # ---- main loop ----
            for s in range(NQT):
                ot_ps = otp.tile([HEAD + 1, QT], F32, tag="ot", name=f"ot_ps{s}")
                prev = []
                for k in range(NKC + 1):
                    cur = []
                    if k < NKC:
                        # chunks 0-3 of sweep 0 run in 512-wide halves: the
                        # h0 half depends only on proj group t0, so exp
                        # starts while xT group t1 is still arriving.
                        split = (s == 0 and k < 4)
                        st = stp.tile([128, QT], F32, tag="st", name=f"st{s}_{k}")
                        p = ppool.tile([128, QT], BF, tag="p", name=f"p{s}_{k}")
                        halves = ((0, 512), (512, 1024)) if split else ((0, QT),)
                        for lo, hi in halves:
                            if lo == 512 and s == 0 and k == 0:
                                # the h1 columns need proj group t1: the qk(1)
                                # wedge must be emitted BEFORE any h1 reader,
                                # or the scores read stale qq on a cold run
                                proj_qk(1)
                            for h in range(lo // 512, hi // 512):
                                gq = slice(s * QT + h * 512, s * QT + (h + 1) * 512)
                                nc.tensor.matmul(
                                    st[:, h * 512:(h + 1) * 512],
                                    kk_ap(k), qq_half(k, gq),
                                    start=True, stop=True,
                                )
                            nc.scalar.activation(
                                p[:, lo:hi], st[:, lo:hi], EXP, scale=ESCALE)
                            gqs = slice(s * QT + lo, s * QT + hi)
                            nc.vector.tensor_mul(
                                p[:, lo:hi], p[:, lo:hi], mt[:, k, gqs])
                            for h in range(lo // 512, hi // 512):
                                cur.append((k, p, h))
                        if s == 0:
                            # remaining projection work wedged into the PE
                            # stream while ACT digests the early chunks
                            if k == 0:
                                proj_vt(0)
                                proj_vtr(0)
                            elif k == 1:
                                proj_vt(1)
                            elif k == 2:
                                proj_vtr(1)
                            elif k == 3:
                                proj_qk(2)
                            elif k == 4:
                                proj_vt(2)
                            elif k == 5:
                                proj_vtr(2)
                            elif k == 6:
                                proj_qk(3)
                            elif k == 7:
                                proj_vt(3)
                            elif k == 8:
                                proj_vtr(3)
                    for (pk, p, h) in prev:
                        nc.tensor.matmul(
                            ot_ps[:, h * 512:(h + 1) * 512],
                            vp[:, pk, :], p[:, h * 512:(h + 1) * 512],
                            start=(pk == 0), stop=(pk == NKC - 1),
                        )
                    prev = cur

                # psum -> sbuf, then DMA out. Sweep 0's copy runs mid-loop
                # where ACT is saturated, so it goes on DVE; sweep 1's copy
                # is split ACT/DVE so both halves finish sooner.
                ot_sb = epool.tile([HEAD + 1, QT], F32, tag="osb", name=f"ot_sb{s}")
                if s == 0:
                    nc.vector.tensor_copy(out=ot_sb, in_=ot_ps)
                    nc.sync.dma_start(out=ot_d[:, 0:QT], in_=ot_sb)
                else:
                    nc.scalar.activation(ot_sb[:, 0:512], ot_ps[:, 0:512], COPY)
                    nc.vector.tensor_copy(
                        out=ot_sb[:, 512:1024], in_=ot_ps[:, 512:1024])
                    nc.sync.dma_start(
                        out=ot_d[:, QT:QT + 512], in_=ot_sb[:, 0:512])
                    nc.sync.dma_start(
                        out=ot_d[:, QT + 512:SEQ], in_=ot_sb[:, 512:1024])

    if os.environ.get('ATTN_HOIST', '1') == '1':
        _hoist_dmas(nc)
    _split_excess_waits(nc)
    _strip_tail(nc)
    return nc


_CACHE = {}


def _get_nc():
    if "nc" not in _CACHE:
        _CACHE["nc"] = _build()
    return _CACHE["nc"]


def _prep_in_maps(x, Wq, bq, Wk, bk, Wv, bv, mask):
    x = np.asarray(x, dtype=np.float32)
    Wqk = np.concatenate(
        [np.asarray(Wq, np.float32), np.asarray(Wk, np.float32)], axis=1)
    # partition-major: row p holds [c0 cols | c1 cols | ...] for w = c*128+p
    Wqkh = np.ascontiguousarray(
        Wqk.reshape(NCH, 128, 128).transpose(1, 0, 2).reshape(128, NCH * 128)
    ).astype(BF16)
    Wvh = np.ascontiguousarray(
        np.asarray(Wv, np.float32).reshape(NCH, 128, HEAD)
        .transpose(1, 0, 2).reshape(128, NCH * HEAD)
    ).astype(BF16)
    bqk = np.concatenate(
        [np.asarray(bq, np.float32), np.asarray(bk, np.float32)]
    ).astype(BF16).reshape(1, 128)
    ident = np.eye(64, dtype=np.float32).astype(BF16)

    # mTh[p, (s*NKC+c)*1024 + j] = mask[s*1024+j, c*128+p]
    mTh = np.ascontiguousarray(
        np.asarray(mask, np.float32).T.reshape(NKC, 128, NQT, QT)
        .transpose(1, 2, 0, 3).reshape(128, NKC * SEQ)
    ).astype(BF16)
    in_maps = []
    for b in range(N_CORES):
        # xth[p, t, c, j] = x[b][t*512+j, c*128+p]
        xth = np.ascontiguousarray(
            x[b].reshape(4, 512, NCH, 128).transpose(3, 0, 2, 1)
            .reshape(128, 4 * NCH * 512)
        ).astype(BF16)
        in_maps.append({
            "xT": xth, "Wqk": Wqkh, "Wv": Wvh, "bqk": bqk, "ident": ident,
            "mT": mTh,
        })
    return in_maps


def _run(in_maps, trace=False, **kw):
    nc = _get_nc()
    return run_bass_kernel_spmd(nc, in_maps, list(range(N_CORES)), trace=trace, **kw)


def kernel(x, Wq, bq, Wk, bk, Wv, bv, mask):
    in_maps = _prep_in_maps(x, Wq, bq, Wk, bk, Wv, bv, mask)
    res = _run(in_maps)
    out = np.empty((N_CORES, SEQ, HEAD), np.float32)
    bvf = np.asarray(bv, np.float32)   # softmax weights sum to 1, so the V
    for b in range(N_CORES):           # bias is just an additive constant
        ot = np.asarray(res.results[b]["ot"])          # [65, 2048] f32
        out[b] = (ot[:HEAD] / ot[HEAD:HEAD + 1]).T + bvf
    return out


# revision 22
# speedup vs baseline: 1.0881x; 1.0679x over previous
"""Single-head attention on 8 Trainium2 NeuronCores (v3).

Problem: x[8, 2048, 768], Wq/Wk/Wv[768, 64]+biases, mask[2048, 2048] int32
Output:  softmax(mask(Q K^T / 8)) V   -> [8, 2048, 64] f32

Sharding: data-parallel over batch - core b computes batch element b.

Per-core dataflow (all matmuls bf16 in / fp32 psum accumulate):
  proj:  QK[128, n] = Wqk.T @ xT per 512-col group; bias added as a k=1
         matmul (lhsT=bias row, rhs=ones row). Q copied to both partition
         halves of qq; K scattered into kk by chunk parity (even chunks at
         partitions 0:64, odd at 64:128) so K needs no duplication for the
         row-tiled score matmuls. V as [keys,64] + ones column -> vp, so the
         PV matmul also yields the softmax denominator for free.
  loop:  per (sweep s of 1024 q-cols, key chunk k): one row-tiled score
         matmul pair -> st psum, one [128,1024] exp on ACT (the critical
         engine: ~33us total at 1 col/cycle), mask multiply on DVE (bf16 2x
         mode), two PV matmuls accumulating OT[65, 1024]; PV for chunk k-1
         is emitted alongside scores for chunk k so the PE stream never
         stalls on the exp/mult latency. Remaining projections are wedged
         into sweep 0's early chunks.
  out:   OT psum -> sbuf -> DMA; host does out[q,h] = OT[h,q]/OT[64,q].

DMAs all ride the SP hardware-DGE queue in consumption order; the mask is
moved in 8 big strided DMAs (4 chunks x 1024 cols each) to keep the issue
cost low (~565ns per DMA on the SP sequencer).
"""

import os

import numpy as np
import ml_dtypes

import bass_rust
import concourse.bass as bass
import concourse.mybir as mybir
import concourse.tile as tile
from concourse.bass_utils import run_bass_kernel_spmd

BF16 = ml_dtypes.bfloat16
F32 = mybir.dt.float32
BF = mybir.dt.bfloat16

N_CORES = 8
SEQ = 2048
WIDTH = 768
HEAD = 64
NCH = WIDTH // 128      # 6 contraction chunks for the projections
NKC = SEQ // 128        # 16 key chunks
QT = 1024               # q tile (columns processed per main-loop sweep)
NQT = SEQ // QT


def _split_excess_waits(nc, max_waits=1):
    """walrus in this container rejects >1 sync wait per instruction; hoist
    extras onto preceding same-engine NoOps (same semantics: the engine
    executes its stream in order, so waiting earlier is equivalent)."""
    n = 0
    for bb in nc.main_func.blocks:
        new_list = []
        for ins in bb.instructions:
            si = ins.sync_info
            if si is not None and len(si.on_wait) > max_waits:
                waits = list(si.on_wait)
                extra, keep = waits[:-max_waits], waits[-max_waits:]
                for j, w in enumerate(extra):
                    nop = bass_rust.InstNoOp(
                        name=f"{ins.name}-ws{j}", engine=ins.engine, ins=[], outs=[]
                    )
                    nop.sync_info = mybir.SyncInfo(on_wait=[w], on_update=[])
                    new_list.append(nop)
                    n += 1
                ins.sync_info = mybir.SyncInfo(
                    on_wait=keep, on_update=list(si.on_update)
                )
            new_list.append(ins)
        bb.instructions = new_list
    return n


def _strip_tail(nc):
    """Drop the NRT pseudo-sync ISA op and the second all-engine barrier that
    TileContext emits after the semaphore reset - ~4-5us of fixed tail. The
    final DMA-drain + first barrier + sem reset are kept, so re-execution of
    the NEFF still starts from clean semaphores."""
    for bb in nc.main_func.blocks:
        ins_list = list(bb.instructions)
        idx = None
        for i, ins in enumerate(ins_list):
            if getattr(ins, "is_reset_sema", False):
                idx = i
        if idx is not None and idx > len(ins_list) - 20:
            bb.instructions = ins_list[:idx + 1]
    return nc




def _hoist_dmas(nc, max_hoist=6):
    """Move wait-free SP-queue input-DMA issues to the front of the program.
    The TileContext/Bass entry sequence (two all-engine barriers, register
    restores) costs ~7.5us before the first DMA would otherwise issue; the
    input DMAs have no dependencies, and per-engine instruction order is all
    that codegen cares about, so issuing them first lets the transfers run
    during the preamble. DMA-completion semaphores only ever increase and
    consumers wait on >= thresholds, so early completion is harmless."""
    blocks = nc.main_func.blocks
    body = None
    for bb in blocks:
        if any(type(ins).__name__ == "InstDMACopy" for ins in bb.instructions):
            body = bb
            break
    if body is None:
        return 0
    total = 0
    for eng, cap in ((mybir.EngineType.SP, max_hoist),
                     (mybir.EngineType.Activation, 2)):
        # leading prefix of wait-free DMAs per engine queue. Only a prefix
        # may move: DMAs sharing a ring semaphore must complete in queue
        # order, so nothing may be hoisted past a waiting DMA.
        front, rest = [], []
        stopped = False
        for ins in body.instructions:
            si = ins.sync_info
            if (not stopped and type(ins).__name__ == "InstDMACopy"
                    and ins.engine == eng and len(front) < cap):
                if si is None or len(si.on_wait) == 0:
                    front.append(ins)
                    continue
                stopped = True
            rest.append(ins)
        if not front:
            continue
        body.instructions = rest
        # insert into the ENTRY block before this engine's first instruction
        # so the transfers run during the runtime preamble.
        entry = blocks[0]
        pos = len(entry.instructions)
        for i, ins in enumerate(entry.instructions):
            if getattr(ins, "engine", None) == eng:
                pos = i
                break
        entry.instructions = (
            entry.instructions[:pos] + front + entry.instructions[pos:]
        )
        total += len(front)
    return total


def _build():
    nc = bass.Bass("TRN2", target_bir_lowering=False, debug=False,
                   num_devices=N_CORES)

    # partition-major host layouts: row p holds everything partition p needs,
    # so each DMA is 128 large contiguous descriptors.
    xT_d = nc.declare_dram_parameter("xT", [128, 4 * NCH * 512], BF, False).ap()
    wqk_d = nc.declare_dram_parameter("Wqk", [128, NCH * 128], BF, False).ap()
    wv_d = nc.declare_dram_parameter("Wv", [128, NCH * HEAD], BF, False).ap()
    bqk_d = nc.declare_dram_parameter("bqk", [1, 128], BF, False).ap()
    id_d = nc.declare_dram_parameter("ident", [64, 64], BF, False).ap()
    # mT layout: [p][(s*NKC + c)*1024 + j] = mask[s*1024+j, c*128+p]
    mT_d = nc.declare_dram_parameter("mT", [128, NKC * SEQ], BF, False).ap()
    ot_d = nc.declare_dram_parameter("ot", [HEAD + 1, SEQ], F32, True).ap()

    EXP = mybir.ActivationFunctionType.Exp
    COPY = mybir.ActivationFunctionType.Copy
    ESCALE = 0.125   # 1/sqrt(HEAD)

    with tile.TileContext(nc) as tc:
        with (
            tc.tile_pool(name="const", bufs=1) as const,
            tc.tile_pool(name="pp", bufs=4) as ppool,
            tc.tile_pool(name="ep", bufs=2) as epool,
            tc.tile_pool(name="stp", bufs=3, space="PSUM") as stp,
            tc.tile_pool(name="otp", bufs=1, space="PSUM") as otp,
        ):
            # ---- input DMAs, all SP queue, in consumption order ----
            wqk = const.tile([128, NCH, 128], BF)
            bqk = const.tile([1, 128], BF)
            wv = const.tile([128, NCH, HEAD], BF)
            xt = const.tile([128, 4, NCH, 512], BF)
            mt = const.tile([128, NQT, NKC, QT], BF)

            def xt_dma(t):
                nc.sync.dma_start(
                    out=xt[:, t, :, :],
                    in_=xT_d[:, t * NCH * 512:(t + 1) * NCH * 512],
                )

            def mask_dma(s, c0, nck, eng=None):
                # chunks c0..c0+nck of sweep s: contiguous in DRAM AND in mt,
                # so each of the 128 descriptor rows moves nck*2KB at once.
                src0 = (s * NKC + c0) * QT
                (eng or nc.sync).dma_start(
                    out=mt[:, s, c0:c0 + nck, :],
                    in_=mT_d[:, src0:src0 + nck * QT],
                )

            ident = const.tile([64, 64], BF)
            # SP queue: weights then xt1, xt2+3; ACT queue (otherwise idle
            # until ~12us): xt0 and the first mask block - both queues start
            # issuing during the runtime preamble via _hoist_dmas.
            nc.sync.dma_start(out=wqk, in_=wqk_d)
            nc.sync.dma_start(out=bqk, in_=bqk_d)
            nc.sync.dma_start(out=ident, in_=id_d)
            nc.scalar.dma_start(
                out=xt[:, 0, :, :], in_=xT_d[:, 0:NCH * 512])
            xt_dma(1)
            nc.sync.dma_start(out=wv, in_=wv_d)
            nc.sync.dma_start(      # xt groups 2+3 as one 12KB-row DMA
                out=xt[:, 2:4, :, :],
                in_=xT_d[:, 2 * NCH * 512:4 * NCH * 512],
            )
            mask_dma(0, 0, 4, eng=nc.scalar)
            mask_dma(0, 4, 4)
            mask_dma(0, 8, 4)
            mask_dma(0, 12, 4)
            for c0 in range(0, NKC, 4):
                mask_dma(1, c0, 4)

            ones = const.tile([1, 512], BF)
            nc.vector.memset(ones, 1.0)

            # ---- projection targets ----
            qq = const.tile([128, SEQ], BF)             # Q on both halves
            kk = const.tile([128, NKC // 2, 128], BF)   # K by chunk parity
            vp = const.tile([128, NKC, HEAD + 1], BF)   # V | ones column
            qktmp = const.tile([128, SEQ], BF)

            def proj_qk(t):
                cols = slice(t * 512, (t + 1) * 512)
                qk_ps = stp.tile([128, 512], F32, tag="st", name=f"qk_ps{t}")
                for c in range(NCH):
                    nc.tensor.matmul(qk_ps, wqk[:, c, :], xt[:, t, c, :],
                                     start=(c == 0), stop=False)
                nc.tensor.matmul(   # +bias: bqk[m] * ones[n]
                    qk_ps, bqk[0:1, :], ones[0:1, 0:512],
                    start=False, stop=True,
                )
                nc.vector.tensor_copy(out=qktmp[:, cols], in_=qk_ps)
                nc.vector.tensor_copy(out=qq[0:64, cols], in_=qktmp[0:64, cols])
                nc.vector.tensor_copy(out=qq[64:128, cols], in_=qktmp[0:64, cols])
                for j in range(4):
                    kc = 4 * t + j          # absolute key chunk
                    half = slice(0, 64) if kc % 2 == 0 else slice(64, 128)
                    nc.vector.tensor_copy(
                        out=kk[half, kc // 2, :],
                        in_=qktmp[64:128, t * 512 + j * 128:t * 512 + (j + 1) * 128],
                    )

            vtmp = const.tile([64, SEQ], BF)    # V^T staging

            def proj_vt(t):
                # V^T[h, keys] for key group t: one [64,512] psum, 6 matmuls
                cols = slice(t * 512, (t + 1) * 512)
                vt_ps = stp.tile([64, 512], F32, tag="st", name=f"vt_ps{t}")
                for c in range(NCH):
                    nc.tensor.matmul(vt_ps, wv[:, c, :], xt[:, t, c, :],
                                     start=(c == 0), stop=(c == NCH - 1))
                nc.vector.tensor_copy(out=vtmp[:, cols], in_=vt_ps)

            def proj_vtr(t):
                # transpose V^T -> V[keys, h] chunks on the PE (bf16 psum)
                tp_ps = stp.tile([128, 4, HEAD], BF, tag="st", name=f"tp_ps{t}")
                for j in range(4):
                    nc.tensor.transpose(
                        tp_ps[:, j, :],
                        vtmp[:, t * 512 + j * 128:t * 512 + (j + 1) * 128],
                        ident)
                lo = 4 * t
                nc.vector.tensor_copy(out=vp[:, lo:lo + 4, 0:HEAD], in_=tp_ps)

            def kk_ap(k):
                half = slice(0, 64) if k % 2 == 0 else slice(64, 128)
                return kk[half, k // 2, :]

            def qq_half(k, gq):
                half = slice(0, 64) if k % 2 == 0 else slice(64, 128)
                return qq[half, gq]

            nc.vector.memset(vp[:, :, HEAD:HEAD + 1], 1.0)
            proj_qk(0)

            # ---- main loop ----
            for s in range(NQT):
                ot_ps = otp.tile([HEAD + 1, QT], F32, tag="ot", name=f"ot_ps{s}")
                prev = []
                for k in range(NKC + 1):
                    cur = []
                    if k < NKC:
                        # chunks 0-3 of sweep 0 run in 512-wide halves: the
                        # h0 half depends only on proj group t0, so exp
                        # starts while xT group t1 is still arriving.
                        split = (s == 0 and k < 4)
                        st = stp.tile([128, QT], F32, tag="st", name=f"st{s}_{k}")
                        p = ppool.tile([128, QT], BF, tag="p", name=f"p{s}_{k}")
                        halves = ((0, 512), (512, 1024)) if split else ((0, QT),)
                        for lo, hi in halves:
                            if lo == 512 and s == 0 and k == 0:
                                # the h1 columns need proj group t1: the qk(1)
                                # wedge must be emitted BEFORE any h1 reader,
                                # or the scores read stale qq on a cold run
                                proj_qk(1)
                            for h in range(lo // 512, hi // 512):
                                gq = slice(s * QT + h * 512, s * QT + (h + 1) * 512)
                                nc.tensor.matmul(
                                    st[:, h * 512:(h + 1) * 512],
                                    kk_ap(k), qq_half(k, gq),
                                    start=True, stop=True,
                                )
                            nc.scalar.activation(
                                p[:, lo:hi], st[:, lo:hi], EXP, scale=ESCALE)
                            nc.vector.tensor_mul(
                                p[:, lo:hi], p[:, lo:hi], mt[:, s, k, lo:hi])
                            for h in range(lo // 512, hi // 512):
                                cur.append((k, p, h))
                        if s == 0:
                            # remaining projection work wedged into the PE
                            # stream while ACT digests the early chunks
                            if k == 0:
                                proj_vt(0)
                                proj_vtr(0)
                            elif k == 1:
                                proj_vt(1)
                            elif k == 2:
                                proj_vtr(1)
                            elif k == 3:
                                proj_qk(2)
                            elif k == 4:
                                proj_vt(2)
                            elif k == 5:
                                proj_vtr(2)
                            elif k == 6:
                                proj_qk(3)
                            elif k == 7:
                                proj_vt(3)
                            elif k == 8:
                                proj_vtr(3)
                    for (pk, p, h) in prev:
                        nc.tensor.matmul(
                            ot_ps[:, h * 512:(h + 1) * 512],
                            vp[:, pk, :], p[:, h * 512:(h + 1) * 512],
                            start=(pk == 0), stop=(pk == NKC - 1),
                        )
                    prev = cur

                # psum -> sbuf, then DMA out. Sweep 0's copy runs mid-loop
                # where ACT is saturated, so it goes on DVE; sweep 1's copy
                # is split ACT/DVE so both halves finish sooner.
                ot_sb = epool.tile([HEAD + 1, QT], F32, tag="osb", name=f"ot_sb{s}")
                if s == 0:
                    nc.vector.tensor_copy(out=ot_sb, in_=ot_ps)
                    nc.sync.dma_start(out=ot_d[:, 0:QT], in_=ot_sb)
                else:
                    nc.scalar.activation(ot_sb[:, 0:512], ot_ps[:, 0:512], COPY)
                    nc.vector.tensor_copy(
                        out=ot_sb[:, 512:1024], in_=ot_ps[:, 512:1024])
                    nc.sync.dma_start(
                        out=ot_d[:, QT:QT + 512], in_=ot_sb[:, 0:512])
                    nc.sync.dma_start(
                        out=ot_d[:, QT + 512:SEQ], in_=ot_sb[:, 512:1024])

    if os.environ.get('ATTN_HOIST', '1') == '1':
        _hoist_dmas(nc)
    _split_excess_waits(nc)
    _strip_tail(nc)
    return nc


_CACHE = {}


def _get_nc():
    if "nc" not in _CACHE:
        _CACHE["nc"] = _build()
    return _CACHE["nc"]


def _prep_in_maps(x, Wq, bq, Wk, bk, Wv, bv, mask):
    x = np.asarray(x, dtype=np.float32)
    Wqk = np.concatenate(
        [np.asarray(Wq, np.float32), np.asarray(Wk, np.float32)], axis=1)
    # partition-major: row p holds [c0 cols | c1 cols | ...] for w = c*128+p
    Wqkh = np.ascontiguousarray(
        Wqk.reshape(NCH, 128, 128).transpose(1, 0, 2).reshape(128, NCH * 128)
    ).astype(BF16)
    Wvh = np.ascontiguousarray(
        np.asarray(Wv, np.float32).reshape(NCH, 128, HEAD)
        .transpose(1, 0, 2).reshape(128, NCH * HEAD)
    ).astype(BF16)
    bqk = np.concatenate(
        [np.asarray(bq, np.float32), np.asarray(bk, np.float32)]
    ).astype(BF16).reshape(1, 128)
    ident = np.eye(64, dtype=np.float32).astype(BF16)

    # mTh[p, (s*NKC+c)*1024 + j] = mask[s*1024+j, c*128+p]
    mTh = np.ascontiguousarray(
        np.asarray(mask, np.float32).T.reshape(NKC, 128, NQT, QT)
        .transpose(1, 2, 0, 3).reshape(128, NKC * SEQ)
    ).astype(BF16)
    in_maps = []
    for b in range(N_CORES):
        # xth[p, t, c, j] = x[b][t*512+j, c*128+p]
        xth = np.ascontiguousarray(
            x[b].reshape(4, 512, NCH, 128).transpose(3, 0, 2, 1)
            .reshape(128, 4 * NCH * 512)
        ).astype(BF16)
        in_maps.append({
            "xT": xth, "Wqk": Wqkh, "Wv": Wvh, "bqk": bqk, "ident": ident,
            "mT": mTh,
        })
    return in_maps


def _run(in_maps, trace=False, **kw):
    nc = _get_nc()
    return run_bass_kernel_spmd(nc, in_maps, list(range(N_CORES)), trace=trace, **kw)


def kernel(x, Wq, bq, Wk, bk, Wv, bv, mask):
    in_maps = _prep_in_maps(x, Wq, bq, Wk, bk, Wv, bv, mask)
    res = _run(in_maps)
    out = np.empty((N_CORES, SEQ, HEAD), np.float32)
    bvf = np.asarray(bv, np.float32)   # softmax weights sum to 1, so the V
    for b in range(N_CORES):           # bias is just an additive constant
        ot = np.asarray(res.results[b]["ot"])          # [65, 2048] f32
        out[b] = (ot[:HEAD] / ot[HEAD:HEAD + 1]).T + bvf
    return out
